# revision 37
# baseline (speedup 1.0000x reference)
"""Trainium2 Bass kernel for nn_Attention_2 (gnn_message_passing).

Pure data parallel over the batch/node dim B=32768: 8 NeuronCores each
process 4096 rows. Per 512-row super-tile, the per-head softmax/gate
pipeline runs in a transposed layout ((h,j) on partitions, b on free
dim) so every reduction is a TensorEngine matmul against tiny
host-built constants (all bf16 for 1 col/cycle streaming).

The memory-dominant aggregation over neighbors is 32 matmuls per
128-row tile with zero-masked [128, 32] bf16 stationary weights (4 live
columns each, packed at 32-col stride in a persistent region tile)
accumulating into 32-row PSUM strips. Context is quantized on the HOST
to fp8(e3m4) with per-(b,k)-row scales — the PE consumes fp8 as the
moving operand directly (1 byte/element on HBM and SBUF), and the bf16
scales are folded into the stationary weights via a precomputed
scale-mask tensor.

The PE is strict in-order, so the serial chain (matmul -> ACT -> matmul
-> DVE -> ...) would stall it between aggregation bursts; instead the
chain matmuls for super-tile t+1 are interleaved between aggregation
strip-groups of super-tile t, giving each ACT/DVE stage a multi-us PE
burst to hide behind. PSUM->SBUF output copies run on the Scalar engine
(GpSimd cannot read PSUM), deferred past the chain's own ACT stages;
output is written bf16 in a [P, NT*D] transposed layout so each
super-tile write is one DMA with 1.5KB-per-partition runs. Context
DMAs are issued two super-tiles ahead, and dependency-free warm-up
matmuls hold the PE's HAM clock gate open through the startup barrier.
"""

import sys

for _p in ("/opt/trn_rl_repo", "/root/.axon_site/_ro/trn_rl_repo"):
    if _p not in sys.path:
        sys.path.insert(0, _p)

from contextlib import ExitStack

import ml_dtypes
import numpy as np

import concourse.bass as bass
import concourse.mybir as mybir
import concourse.tile as tile
from concourse import bacc
from concourse.bass_utils import run_bass_kernel_spmd

# Problem shape (hardcoded; kernel.py must be self-contained)
B, K, D, H = 32768, 32, 192, 4
NCORES = 8
ROWS = B // NCORES          # 4096 rows per core
P = 128                     # partitions / rows per tile
NT = ROWS // P              # 32 tiles per core
G = 4                       # rows per aggregation block (G*K == P)
NB = P // G                 # 32 blocks per tile
NS = P // 32                # 4 output strips (32 rows each) per tile
BPS = NB // NS              # 8 blocks per strip
HK = H * K                  # 128
ST = 4                      # tiles per super-tile (512-row softmax/gate chain)
SP = ST * P                 # 512
NREG = 2 * ST               # stationary regions (double-buffered per half)
REGW = NB * 32              # 1024: packed 32-col stationary buffers

F32 = mybir.dt.float32
BF16 = mybir.dt.bfloat16

_CACHE: dict = {}


def build_program(nt: int = NT):
    rows = nt * P
    nc = bacc.Bacc("TRN2", target_bir_lowering=False, debug=False, num_devices=NCORES)

    # Host-pretransposed inputs: sd as bf16 [K, rows]; ctx as bf16
    # [P, nt*NB*D] with ctx_host[p, (t, j, d)] = context[128 t + 4 j + p//K,
    # p%K, d] — every per-half-tile DMA reads one contiguous 12KB run per
    # partition.
    sd_d = nc.dram_tensor("sd", [K, rows], BF16, kind="ExternalInput").ap()
    ctx_d = nc.dram_tensor("ctx", [P, nt * NB * D], mybir.dt.float8e3,
                           kind="ExternalInput").ap()
    kern_d = nc.dram_tensor("kern_r", [K, HK], BF16, kind="ExternalInput").ap()
    biases_d = nc.dram_tensor("biases_c", [HK, 1], F32, kind="ExternalInput").ap()
    blkones_d = nc.dram_tensor("blkones", [HK, H], BF16, kind="ExternalInput").ap()
    e4_d = nc.dram_tensor("e4", [H, HK], BF16, kind="ExternalInput").ap()
    gd_d = nc.dram_tensor("gd", [HK, HK], BF16, kind="ExternalInput").ap()
    gatebh_d = nc.dram_tensor("gatebh", [HK, 1], F32, kind="ExternalInput").ap()
    hg4h_d = nc.dram_tensor("hg4h", [HK, P], BF16, kind="ExternalInput").ap()
    # per-row int8 scales pre-multiplied by the block mask:
    # smask[p, g] = (p//32 == g%4) * s[g, p%32]  (g = global row on this core)
    smask_d = nc.dram_tensor("smask", [P, rows], BF16, kind="ExternalInput").ap()
    # out[p, t*D + d] = result[128 t + p, d] (bf16; host casts back to f32)
    out_d = nc.dram_tensor("out", [P, nt * D], BF16, kind="ExternalOutput").ap()

    with tile.TileContext(nc) as tc, ExitStack() as ctx:
        consts = ctx.enter_context(tc.tile_pool(name="consts", bufs=1))
        sdp = ctx.enter_context(tc.tile_pool(name="sdp", bufs=3))
        ctbp = ctx.enter_context(tc.tile_pool(name="ctbp", bufs=4 * ST))
        smallp = ctx.enter_context(tc.tile_pool(name="smallp", bufs=3))
        outp = ctx.enter_context(tc.tile_pool(name="outp", bufs=3))
        ps_ch = ctx.enter_context(tc.tile_pool(name="ps_ch", bufs=1, space="PSUM"))
        ps_out = ctx.enter_context(tc.tile_pool(name="ps_out", bufs=3, space="PSUM"))

        c_kern = consts.tile([K, HK], BF16)
        nc.sync.dma_start(c_kern[:], kern_d)
        c_bias = consts.tile([HK, 1], F32)
        nc.sync.dma_start(c_bias[:], biases_d)
        c_blk = consts.tile([HK, H], BF16)
        nc.sync.dma_start(c_blk[:], blkones_d)
        c_e4 = consts.tile([H, HK], BF16)
        nc.sync.dma_start(c_e4[:], e4_d)
        c_gd = consts.tile([HK, HK], BF16)
        nc.sync.dma_start(c_gd[:], gd_d)
        c_gbh = consts.tile([HK, 1], F32)
        nc.sync.dma_start(c_gbh[:], gatebh_d)
        c_hg = consts.tile([HK, P], BF16)
        nc.sync.dma_start(c_hg[:], hg4h_d)

        # Stationary-weight regions: 32 packed buffers of 32 bf16 columns;
        # buffer j's only nonzero columns are its live 4 (within-buffer offset
        # 4*(j%8)), rewritten per tile. The rest stays zero from a one-time
        # memset.
        regions = [consts.tile([P, REGW], BF16, name=f"agg_region{ri}")
                   for ri in range(NREG)]
        for reg0 in regions:
            nc.gpsimd.memset(reg0[:], 0.0)

        # [128, 8, 4] views: live columns of strip s sit at 256 s + 36 j'' + x
        def reg_strip_view(reg, s):
            return reg[:, 256 * s:256 * s + 256].rearrange(
                "p (j x) -> p j x", x=G)[:, 0:BPS * 9 - 8:9, :]

        assert nt % ST == 0
        sts = list(range(0, nt, ST))

        def emit_ctb(t):
            # one tile + DMA per 128-row half so buffers recycle at half
            # granularity; fp8(e3m4) context feeds the PE moving operand
            # directly, so HBM and SBUF both carry 1 byte/element
            halves = []
            HB = NB * D // 2
            for hh in range(ST):
                cb = ctbp.tile([P, NB * D], mybir.dt.float8e3, tag="ctb",
                               name=f"ctb_{t}_{hh}")
                c_base = (t + hh) * NB * D
                # two DMAs per half so the first/last strips can start earlier
                nc.sync.dma_start(cb[:, 0:HB], ctx_d[:, c_base:c_base + HB])
                nc.sync.dma_start(cb[:, HB:], ctx_d[:, c_base + HB:c_base + 2 * HB])
                halves.append(cb)
            return halves

        def chain_stages(t):
            """Generator: yields after each PE stage of the softmax/gate chain
            for the super-tile starting at tile t, so the caller can interleave
            aggregation matmuls of the previous super-tile between stages."""
            r0 = t * P

            sd_t = sdp.tile([K, SP], BF16)
            nc.scalar.dma_start(sd_t[:], sd_d[:, r0:r0 + SP])
            sm_t = sdp.tile([P, SP], BF16, tag="sm")
            nc.scalar.dma_start(sm_t[:], smask_d[:, r0:r0 + SP])

            # simi_T = exp(-0.5 * sd^2) in [K, SP] layout
            sq = smallp.tile([K, SP], F32, tag="sq")
            nc.vector.tensor_mul(sq[:], sd_t[:], sd_t[:])
            simi_T = smallp.tile([K, SP], BF16, tag="simi")
            nc.scalar.activation(simi_T[:], sq[:],
                                 mybir.ActivationFunctionType.Exp, scale=-0.5)

            # logits_T[(h,j), b] then p = exp(logits + bias)
            logits_ps = ps_ch.tile([HK, SP], F32, tag="logits")
            nc.tensor.matmul(logits_ps[:], lhsT=c_kern[:], rhs=simi_T[:])
            yield
            p_t = smallp.tile([HK, SP], BF16, tag="p")
            nc.scalar.activation(p_t[:], logits_ps[:],
                                 mybir.ActivationFunctionType.Exp, bias=c_bias[:])

            # per-(h,b) softmax denominator and its reciprocal, broadcast back
            s_ps = ps_ch.tile([H, SP], F32, tag="s")
            nc.tensor.matmul(s_ps[:], lhsT=c_blk[:], rhs=p_t[:])
            yield
            rs32 = smallp.tile([H, SP], F32, tag="rs32")
            nc.vector.reciprocal_approx_fast(out=rs32[:], in_=s_ps[:])
            rs = smallp.tile([H, SP], BF16, tag="rs")
            nc.vector.tensor_copy(rs[:], rs32[:])
            sbc_ps = ps_ch.tile([HK, SP], F32, tag="sbc")
            nc.tensor.matmul(sbc_ps[:], lhsT=c_e4[:], rhs=rs[:])
            yield
            w_t = smallp.tile([HK, SP], BF16, tag="w")
            nc.vector.tensor_mul(w_t[:], p_t[:], sbc_ps[:])

            # gate: sigmoid(x) = 0.5*(1+tanh(x/2)); the 0.5 is folded into hg4h
            gl_ps = ps_ch.tile([HK, SP], F32, tag="gl")
            nc.tensor.matmul(gl_ps[:], lhsT=c_gd[:], rhs=w_t[:])
            yield
            th = smallp.tile([HK, SP], F32, tag="th")
            nc.scalar.activation(th[:], gl_ps[:],
                                 mybir.ActivationFunctionType.Tanh,
                                 bias=c_gbh[:], scale=0.5)
            gated2 = smallp.tile([HK, SP], BF16, tag="g2")
            nc.vector.scalar_tensor_tensor(
                out=gated2[:], in0=th[:], scalar=1.0, in1=w_t[:],
                op0=mybir.AluOpType.add, op1=mybir.AluOpType.mult)

            # head-combine (replicated 4x over row-groups), then block-mask the
            # live columns straight into each half-tile's stationary region
            wrep_ps = ps_ch.tile([P, SP], F32, tag="wrep")
            nc.tensor.matmul(wrep_ps[:], lhsT=c_hg[:], rhs=gated2[:])
            for hh in range(ST):
                reg = regions[(t + hh) % NREG]
                for s in range(NS):
                    c0 = hh * P + 32 * s
                    wview = wrep_ps[:, c0:c0 + 32].rearrange("p (j x) -> p j x", x=G)
                    smview = sm_t[:, c0:c0 + 32].rearrange("p (j x) -> p j x", x=G)
                    nc.vector.tensor_mul(reg_strip_view(reg, s), wview, smview)
            yield

        def emit_agg_strips(t, hh, s_lo, s_hi, ctb, out_ps):
            """Aggregation strips [s_lo, s_hi) of half-tile hh: per 32-row
            strip, 8 PSUM-accumulating matmuls with 32-col stationaries."""
            reg = regions[(t + hh) % NREG]
            cb = ctb[hh]
            for s in range(s_lo, s_hi):
                for jj in range(BPS):
                    j = s * BPS + jj
                    nc.tensor.matmul(
                        out_ps[32 * s:32 * s + 32, :],
                        lhsT=reg[:, 256 * s + 32 * jj:256 * s + 32 * jj + 32],
                        rhs=cb[:, j * D:(j + 1) * D],
                        start=(jj == 0), stop=(jj == BPS - 1),
                        tile_position=(0, 32 * s))

        # HAM warm-up: dependency-free dummy matmuls keep the PE active during
        # the startup barrier + first chain, so the clock gate is at 8/8 when
        # real aggregation work arrives (~3.4us of activity required)
        warm_ps = ps_out.tile([P, D], F32, tag="outps", name="warm_ps")

        def warm(n):
            for _ in range(n):
                nc.tensor.matmul(warm_ps[0:32, 0:64], lhsT=c_kern[:, 0:32],
                                 rhs=c_kern[:, 64:128], tile_position=(0, 0))

        warm(30)

        # prologue: context DMAs two super-tiles ahead; first chain with
        # warm-up matmuls as PE filler between its serial stages
        ctbs = {s: emit_ctb(s) for s in sts[:2]}
        g0 = chain_stages(sts[0])
        while next(g0, StopIteration) is not StopIteration:
            warm(8)

        for i, t in enumerate(sts):
            if i + 2 < len(sts):
                ctbs[sts[i + 2]] = emit_ctb(sts[i + 2])
            nxt = chain_stages(sts[i + 1]) if i + 1 < len(sts) else iter(())
            ctb = ctbs.pop(t)
            out_sb = outp.tile([P, ST * D], BF16)

            # interleave chain(t+1) PE stages between aggregation bursts of t;
            # each DVE/ACT stage gets a multi-us PE burst to complete under
            # PSUM->SBUF copies run on Scalar (ACT reads PSUM; GpSimd cannot)
            # and are deferred past the chain's own ACT stages so the chain
            # never queues behind an aggregation-dependent copy.
            def copy_half(hh):
                nc.scalar.activation(out_sb[:, hh * D:(hh + 1) * D],
                                     out_ps[hh][:],
                                     mybir.ActivationFunctionType.Copy)

            out_ps = {}
            for hh in range(ST):
                out_ps[hh] = ps_out.tile([P, D], F32, tag="outps",
                                         name=f"outps_{t}_{hh}")
            next(nxt, None)                        # logits(t+1)
            emit_agg_strips(t, 0, 0, NS, ctb, out_ps[0])
            next(nxt, None)                        # exp, s-sum(t+1)
            emit_agg_strips(t, 1, 0, NS, ctb, out_ps[1])
            next(nxt, None)                        # recip, sbc(t+1)
            emit_agg_strips(t, 2, 0, 2, ctb, out_ps[2])
            next(nxt, None)                        # w_t, gl(t+1)
            emit_agg_strips(t, 2, 2, NS, ctb, out_ps[2])
            next(nxt, None)                        # tanh, gated2, wrep(t+1), regions
            copy_half(0)
            copy_half(1)
            copy_half(2)
            emit_agg_strips(t, 3, 0, NS, ctb, out_ps[3])
            next(nxt, None)                        # (exhaust)
            copy_half(3)
            nc.sync.dma_start(out_d[:, t * D:(t + ST) * D], out_sb[:])

    nc.compile()
    return nc


def _softmax(x):
    e = np.exp(x - x.max())
    return e / e.sum()


def _to_bf16(a):
    """Fast vectorized float32 -> bfloat16 with round-to-nearest-even."""
    a = np.ascontiguousarray(np.asarray(a, np.float32))
    v = a.view(np.uint32)
    r = (v >> 16) & np.uint32(1)
    out = ((v + np.uint32(0x7FFF) + r) >> 16).astype(np.uint16)
    return out.view(ml_dtypes.bfloat16)


def build_consts(kernels, biases, gate_W, gate_b, gate_weights, gate_bias):
    f32 = np.float32
    bf16 = ml_dtypes.bfloat16
    kern_r = np.ascontiguousarray(
        kernels.transpose(1, 0, 2).reshape(K, HK)).astype(bf16)
    biases_c = np.ascontiguousarray(biases.reshape(HK, 1)).astype(f32)
    blkones = np.kron(np.eye(H), np.ones((K, 1))).astype(bf16)
    e4 = np.kron(np.eye(H), np.ones((1, K))).astype(bf16)
    gd = np.kron(np.eye(H), gate_W).astype(bf16)
    gatebh = (0.5 * np.tile(gate_b, H)).reshape(HK, 1).astype(f32)
    hg = _softmax(np.asarray(gate_weights, np.float64) + np.asarray(gate_bias, np.float64))
    hg4h = np.kron((0.5 * hg)[:, None] @ np.ones((1, H)), np.eye(K)).astype(bf16)
    return dict(kern_r=kern_r, biases_c=biases_c, blkones=blkones, e4=e4, gd=gd,
                gatebh=gatebh, hg4h=hg4h)


def run(inputs: dict, trace: bool = False, **kw):
    """inputs: full-size arrays keyed as in setup_inputs(). Returns (out, results)."""
    if "nc" not in _CACHE:
        _CACHE["nc"] = build_program()
    nc = _CACHE["nc"]

    sd16 = _to_bf16(inputs["source_distance"])   # [B, K] bf16
    # fp8(e3m4) quantization of context with per-(b,k)-row scales (rowmax
    # mapped to 14.0, under the e3m4 max of 15.5); the bf16 scales are folded
    # into the aggregation weights on-device
    ctx = np.ascontiguousarray(np.asarray(inputs["context"], np.float32))
    scales = np.maximum(np.abs(ctx).max(axis=2) / 14.0, 1e-12)    # [B, K]
    s16 = scales.astype(ml_dtypes.bfloat16)                       # [B, K]
    ctxq = (ctx * (1.0 / s16.astype(np.float32))[:, :, None]
            ).astype(ml_dtypes.float8_e3m4)                       # [B, K, D]
    consts = build_consts(
        np.asarray(inputs["kernels"], np.float32),
        np.asarray(inputs["biases"], np.float32),
        np.asarray(inputs["gate_W"], np.float32),
        np.asarray(inputs["gate_b"], np.float32),
        np.asarray(inputs["gate_weights"], np.float32),
        np.asarray(inputs["gate_bias"], np.float32),
    )

    maskpat = (np.arange(P)[:, None] // K == np.arange(ROWS)[None, :] % G)
    prow = np.arange(P) % K
    in_maps = []
    for c in range(NCORES):
        b0 = c * ROWS
        # host-side layout transforms so every device DMA run is long+contiguous
        sd_c = np.ascontiguousarray(sd16[b0:b0 + ROWS].T)                  # [K, ROWS]
        ctx_c = np.ascontiguousarray(
            ctxq[b0:b0 + ROWS].reshape(NT, NB, P, D).transpose(2, 0, 1, 3)
        ).reshape(P, NT * NB * D)
        # smask[p, g] = (p//32 == g%4) * s[b0+g, p%32]
        sm_c = np.where(maskpat, s16[b0:b0 + ROWS].T[prow], ml_dtypes.bfloat16(0))
        m = {"sd": sd_c, "ctx": ctx_c, "smask": np.ascontiguousarray(sm_c)}
        m.update(consts)
        in_maps.append(m)

    results = run_bass_kernel_spmd(nc, in_maps, core_ids=list(range(NCORES)),
                                   trace=trace, **kw)
    outs = []
    for c in range(NCORES):
        o = np.asarray(results.results[c]["out"]).astype(np.float32)
        outs.append(o.reshape(P, NT, D).transpose(1, 0, 2).reshape(ROWS, D))
    out = np.concatenate(outs, axis=0)
    return out, results


def kernel(**inputs) -> np.ndarray:
    out, _ = run(inputs)
    return out


# revision 38
# speedup vs baseline: 1.0160x; 1.0160x over previous
"""Trainium2 Bass kernel for nn_Attention_2 (gnn_message_passing).

Pure data parallel over the batch/node dim B=32768: 8 NeuronCores each
process 4096 rows. Per 512-row super-tile, the per-head softmax/gate
pipeline runs in a transposed layout ((h,j) on partitions, b on free
dim) so every reduction is a TensorEngine matmul against tiny
host-built constants (all bf16 for 1 col/cycle streaming).

The memory-dominant aggregation over neighbors is 32 matmuls per
128-row tile with zero-masked [128, 32] bf16 stationary weights (4 live
columns each, packed at 32-col stride in a persistent region tile)
accumulating into 32-row PSUM strips. Context is quantized on the HOST
to fp8(e3m4) with per-(b,k)-row scales — the PE consumes fp8 as the
moving operand directly (1 byte/element on HBM and SBUF), and the bf16
scales are folded into the stationary weights via a precomputed
scale-mask tensor.

The PE is strict in-order, so the serial chain (matmul -> ACT -> matmul
-> DVE -> ...) would stall it between aggregation bursts; instead the
chain matmuls for super-tile t+1 are interleaved between aggregation
strip-groups of super-tile t, giving each ACT/DVE stage a multi-us PE
burst to hide behind. PSUM->SBUF output copies run on the Scalar engine
(GpSimd cannot read PSUM), deferred past the chain's own ACT stages;
output is written bf16 in a [P, NT*D] transposed layout so each
super-tile write is one DMA with 1.5KB-per-partition runs. Context
DMAs are issued two super-tiles ahead, and dependency-free warm-up
matmuls hold the PE's HAM clock gate open through the startup barrier.
"""

import sys

for _p in ("/opt/trn_rl_repo", "/root/.axon_site/_ro/trn_rl_repo"):
    if _p not in sys.path:
        sys.path.insert(0, _p)

from contextlib import ExitStack

import ml_dtypes
import numpy as np

import concourse.bass as bass
import concourse.mybir as mybir
import concourse.tile as tile
from concourse import bacc
from concourse.bass_utils import run_bass_kernel_spmd

# Problem shape (hardcoded; kernel.py must be self-contained)
B, K, D, H = 32768, 32, 192, 4
NCORES = 8
ROWS = B // NCORES          # 4096 rows per core
P = 128                     # partitions / rows per tile
NT = ROWS // P              # 32 tiles per core
G = 4                       # rows per aggregation block (G*K == P)
NB = P // G                 # 32 blocks per tile
NS = P // 32                # 4 output strips (32 rows each) per tile
BPS = NB // NS              # 8 blocks per strip
HK = H * K                  # 128
ST = 4                      # tiles per super-tile (512-row softmax/gate chain)
SP = ST * P                 # 512
NREG = 2 * ST               # stationary regions (double-buffered per half)
REGW = NB * 32              # 1024: packed 32-col stationary buffers

F32 = mybir.dt.float32
BF16 = mybir.dt.bfloat16

_CACHE: dict = {}


def build_program(nt: int = NT):
    rows = nt * P
    nc = bacc.Bacc("TRN2", target_bir_lowering=False, debug=False, num_devices=NCORES)

    # Host-pretransposed inputs: sd as bf16 [K, rows]; ctx as bf16
    # [P, nt*NB*D] with ctx_host[p, (t, j, d)] = context[128 t + 4 j + p//K,
    # p%K, d] — every per-half-tile DMA reads one contiguous 12KB run per
    # partition.
    sd_d = nc.dram_tensor("sd", [K, rows], BF16, kind="ExternalInput").ap()
    ctx_d = nc.dram_tensor("ctx", [P, nt * NB * D], mybir.dt.float8e3,
                           kind="ExternalInput").ap()
    kern_d = nc.dram_tensor("kern_r", [K, HK], BF16, kind="ExternalInput").ap()
    biases_d = nc.dram_tensor("biases_c", [HK, 1], F32, kind="ExternalInput").ap()
    blkones_d = nc.dram_tensor("blkones", [HK, H], BF16, kind="ExternalInput").ap()
    e4_d = nc.dram_tensor("e4", [H, HK], BF16, kind="ExternalInput").ap()
    gd_d = nc.dram_tensor("gd", [HK, HK], BF16, kind="ExternalInput").ap()
    gatebh_d = nc.dram_tensor("gatebh", [HK, 1], F32, kind="ExternalInput").ap()
    hg4h_d = nc.dram_tensor("hg4h", [HK, P], BF16, kind="ExternalInput").ap()
    # per-row int8 scales pre-multiplied by the block mask:
    # smask[p, g] = (p//32 == g%4) * s[g, p%32]  (g = global row on this core)
    smask_d = nc.dram_tensor("smask", [P, rows], BF16, kind="ExternalInput").ap()
    # out[p, t*D + d] = result[128 t + p, d] (bf16; host casts back to f32)
    out_d = nc.dram_tensor("out", [P, nt * D], BF16, kind="ExternalOutput").ap()

    with tile.TileContext(nc) as tc, ExitStack() as ctx:
        consts = ctx.enter_context(tc.tile_pool(name="consts", bufs=1))
        sdp = ctx.enter_context(tc.tile_pool(name="sdp", bufs=3))
        ctbp = ctx.enter_context(tc.tile_pool(name="ctbp", bufs=4 * ST))
        smallp = ctx.enter_context(tc.tile_pool(name="smallp", bufs=3))
        outp = ctx.enter_context(tc.tile_pool(name="outp", bufs=3))
        ps_ch = ctx.enter_context(tc.tile_pool(name="ps_ch", bufs=1, space="PSUM"))
        ps_out = ctx.enter_context(tc.tile_pool(name="ps_out", bufs=3, space="PSUM"))

        c_kern = consts.tile([K, HK], BF16)
        nc.sync.dma_start(c_kern[:], kern_d)
        c_bias = consts.tile([HK, 1], F32)
        nc.sync.dma_start(c_bias[:], biases_d)
        c_blk = consts.tile([HK, H], BF16)
        nc.sync.dma_start(c_blk[:], blkones_d)
        c_e4 = consts.tile([H, HK], BF16)
        nc.sync.dma_start(c_e4[:], e4_d)
        c_gd = consts.tile([HK, HK], BF16)
        nc.sync.dma_start(c_gd[:], gd_d)
        c_gbh = consts.tile([HK, 1], F32)
        nc.sync.dma_start(c_gbh[:], gatebh_d)
        c_hg = consts.tile([HK, P], BF16)
        nc.sync.dma_start(c_hg[:], hg4h_d)

        # Stationary-weight regions: 32 packed buffers of 32 bf16 columns;
        # buffer j's only nonzero columns are its live 4 (within-buffer offset
        # 4*(j%8)), rewritten per tile. The rest stays zero from a one-time
        # memset.
        regions = [consts.tile([P, REGW], BF16, name=f"agg_region{ri}")
                   for ri in range(NREG)]
        for reg0 in regions:
            nc.gpsimd.memset(reg0[:], 0.0)

        # [128, 8, 4] views: live columns of strip s sit at 256 s + 36 j'' + x
        def reg_strip_view(reg, s):
            return reg[:, 256 * s:256 * s + 256].rearrange(
                "p (j x) -> p j x", x=G)[:, 0:BPS * 9 - 8:9, :]

        assert nt % ST == 0
        sts = list(range(0, nt, ST))

        def emit_ctb(t):
            # one tile + DMA per 128-row half so buffers recycle at half
            # granularity; fp8(e3m4) context feeds the PE moving operand
            # directly, so HBM and SBUF both carry 1 byte/element
            halves = []
            HB = NB * D // 2
            for hh in range(ST):
                cb = ctbp.tile([P, NB * D], mybir.dt.float8e3, tag="ctb",
                               name=f"ctb_{t}_{hh}")
                c_base = (t + hh) * NB * D
                # two DMAs per half so the first/last strips can start earlier
                nc.sync.dma_start(cb[:, 0:HB], ctx_d[:, c_base:c_base + HB])
                nc.sync.dma_start(cb[:, HB:], ctx_d[:, c_base + HB:c_base + 2 * HB])
                halves.append(cb)
            return halves

        def chain_stages(t):
            """Generator: yields after each PE stage of the softmax/gate chain
            for the super-tile starting at tile t, so the caller can interleave
            aggregation matmuls of the previous super-tile between stages."""
            r0 = t * P

            sd_t = sdp.tile([K, SP], BF16)
            nc.scalar.dma_start(sd_t[:], sd_d[:, r0:r0 + SP])
            sm_t = sdp.tile([P, SP], BF16, tag="sm")
            nc.scalar.dma_start(sm_t[:], smask_d[:, r0:r0 + SP])

            # simi_T = exp(-0.5 * sd^2) in [K, SP] layout
            sq = smallp.tile([K, SP], F32, tag="sq")
            nc.vector.tensor_mul(sq[:], sd_t[:], sd_t[:])
            simi_T = smallp.tile([K, SP], BF16, tag="simi")
            nc.scalar.activation(simi_T[:], sq[:],
                                 mybir.ActivationFunctionType.Exp, scale=-0.5)

            # logits_T[(h,j), b] then p = exp(logits + bias)
            logits_ps = ps_ch.tile([HK, SP], F32, tag="logits")
            nc.tensor.matmul(logits_ps[:], lhsT=c_kern[:], rhs=simi_T[:])
            yield
            p_t = smallp.tile([HK, SP], BF16, tag="p")
            nc.scalar.activation(p_t[:], logits_ps[:],
                                 mybir.ActivationFunctionType.Exp, bias=c_bias[:])

            # per-(h,b) softmax denominator and its reciprocal, broadcast back
            s_ps = ps_ch.tile([H, SP], F32, tag="s")
            nc.tensor.matmul(s_ps[:], lhsT=c_blk[:], rhs=p_t[:])
            yield
            rs32 = smallp.tile([H, SP], F32, tag="rs32")
            nc.vector.reciprocal_approx_fast(out=rs32[:], in_=s_ps[:])
            rs = smallp.tile([H, SP], BF16, tag="rs")
            nc.vector.tensor_copy(rs[:], rs32[:])
            sbc_ps = ps_ch.tile([HK, SP], F32, tag="sbc")
            nc.tensor.matmul(sbc_ps[:], lhsT=c_e4[:], rhs=rs[:])
            yield
            w_t = smallp.tile([HK, SP], BF16, tag="w")
            nc.vector.tensor_mul(w_t[:], p_t[:], sbc_ps[:])

            # gate: sigmoid(x) = 0.5*(1+tanh(x/2)); the 0.5 is folded into hg4h
            gl_ps = ps_ch.tile([HK, SP], F32, tag="gl")
            nc.tensor.matmul(gl_ps[:], lhsT=c_gd[:], rhs=w_t[:])
            yield
            th = smallp.tile([HK, SP], F32, tag="th")
            nc.scalar.activation(th[:], gl_ps[:],
                                 mybir.ActivationFunctionType.Tanh,
                                 bias=c_gbh[:], scale=0.5)
            gated2 = smallp.tile([HK, SP], BF16, tag="g2")
            nc.vector.scalar_tensor_tensor(
                out=gated2[:], in0=th[:], scalar=1.0, in1=w_t[:],
                op0=mybir.AluOpType.add, op1=mybir.AluOpType.mult)

            # head-combine (replicated 4x over row-groups), then block-mask the
            # live columns straight into each half-tile's stationary region
            wrep_ps = ps_ch.tile([P, SP], F32, tag="wrep")
            nc.tensor.matmul(wrep_ps[:], lhsT=c_hg[:], rhs=gated2[:])
            for hh in range(ST):
                reg = regions[(t + hh) % NREG]
                for s in range(NS):
                    c0 = hh * P + 32 * s
                    wview = wrep_ps[:, c0:c0 + 32].rearrange("p (j x) -> p j x", x=G)
                    smview = sm_t[:, c0:c0 + 32].rearrange("p (j x) -> p j x", x=G)
                    nc.vector.tensor_mul(reg_strip_view(reg, s), wview, smview)
            yield

        def emit_agg_strips(t, hh, s_lo, s_hi, ctb, out_ps):
            """Aggregation strips [s_lo, s_hi) of half-tile hh: per 32-row
            strip, 8 PSUM-accumulating matmuls with 32-col stationaries."""
            reg = regions[(t + hh) % NREG]
            cb = ctb[hh]
            for s in range(s_lo, s_hi):
                for jj in range(BPS):
                    j = s * BPS + jj
                    nc.tensor.matmul(
                        out_ps[32 * s:32 * s + 32, :],
                        lhsT=reg[:, 256 * s + 32 * jj:256 * s + 32 * jj + 32],
                        rhs=cb[:, j * D:(j + 1) * D],
                        start=(jj == 0), stop=(jj == BPS - 1),
                        tile_position=(0, 32 * s))

        # HAM warm-up: dependency-free dummy matmuls keep the PE active during
        # the startup barrier + first chain, so the clock gate is at 8/8 when
        # real aggregation work arrives (~3.4us of activity required)
        warm_ps = ps_out.tile([P, D], F32, tag="outps", name="warm_ps")

        def warm(n):
            for _ in range(n):
                nc.tensor.matmul(warm_ps[0:32, 0:64], lhsT=c_kern[:, 0:32],
                                 rhs=c_kern[:, 64:128], tile_position=(0, 0))

        warm(30)

        # prologue: context DMAs two super-tiles ahead; first chain with
        # warm-up matmuls as PE filler between its serial stages
        ctbs = {s: emit_ctb(s) for s in sts[:2]}
        g0 = chain_stages(sts[0])
        while next(g0, StopIteration) is not StopIteration:
            warm(3)

        for i, t in enumerate(sts):
            if i + 2 < len(sts):
                ctbs[sts[i + 2]] = emit_ctb(sts[i + 2])
            nxt = chain_stages(sts[i + 1]) if i + 1 < len(sts) else iter(())
            ctb = ctbs.pop(t)
            out_sb = outp.tile([P, ST * D], BF16)

            # interleave chain(t+1) PE stages between aggregation bursts of t;
            # each DVE/ACT stage gets a multi-us PE burst to complete under
            # PSUM->SBUF copies run on Scalar (ACT reads PSUM; GpSimd cannot)
            # and are deferred past the chain's own ACT stages so the chain
            # never queues behind an aggregation-dependent copy.
            def copy_half(hh):
                nc.scalar.activation(out_sb[:, hh * D:(hh + 1) * D],
                                     out_ps[hh][:],
                                     mybir.ActivationFunctionType.Copy)

            out_ps = {}
            for hh in range(ST):
                out_ps[hh] = ps_out.tile([P, D], F32, tag="outps",
                                         name=f"outps_{t}_{hh}")
            next(nxt, None)                        # logits(t+1)
            emit_agg_strips(t, 0, 0, NS, ctb, out_ps[0])
            next(nxt, None)                        # exp, s-sum(t+1)
            emit_agg_strips(t, 1, 0, NS, ctb, out_ps[1])
            next(nxt, None)                        # recip, sbc(t+1)
            emit_agg_strips(t, 2, 0, 2, ctb, out_ps[2])
            next(nxt, None)                        # w_t, gl(t+1)
            emit_agg_strips(t, 2, 2, NS, ctb, out_ps[2])
            next(nxt, None)                        # tanh, gated2, wrep(t+1), regions
            copy_half(0)
            copy_half(1)
            copy_half(2)
            emit_agg_strips(t, 3, 0, NS, ctb, out_ps[3])
            next(nxt, None)                        # (exhaust)
            copy_half(3)
            nc.sync.dma_start(out_d[:, t * D:(t + ST) * D], out_sb[:])

    nc.compile()
    return nc


def _softmax(x):
    e = np.exp(x - x.max())
    return e / e.sum()


def _to_bf16(a):
    """Fast vectorized float32 -> bfloat16 with round-to-nearest-even."""
    a = np.ascontiguousarray(np.asarray(a, np.float32))
    v = a.view(np.uint32)
    r = (v >> 16) & np.uint32(1)
    out = ((v + np.uint32(0x7FFF) + r) >> 16).astype(np.uint16)
    return out.view(ml_dtypes.bfloat16)


def build_consts(kernels, biases, gate_W, gate_b, gate_weights, gate_bias):
    f32 = np.float32
    bf16 = ml_dtypes.bfloat16
    kern_r = np.ascontiguousarray(
        kernels.transpose(1, 0, 2).reshape(K, HK)).astype(bf16)
    biases_c = np.ascontiguousarray(biases.reshape(HK, 1)).astype(f32)
    blkones = np.kron(np.eye(H), np.ones((K, 1))).astype(bf16)
    e4 = np.kron(np.eye(H), np.ones((1, K))).astype(bf16)
    gd = np.kron(np.eye(H), gate_W).astype(bf16)
    gatebh = (0.5 * np.tile(gate_b, H)).reshape(HK, 1).astype(f32)
    hg = _softmax(np.asarray(gate_weights, np.float64) + np.asarray(gate_bias, np.float64))
    hg4h = np.kron((0.5 * hg)[:, None] @ np.ones((1, H)), np.eye(K)).astype(bf16)
    return dict(kern_r=kern_r, biases_c=biases_c, blkones=blkones, e4=e4, gd=gd,
                gatebh=gatebh, hg4h=hg4h)


def run(inputs: dict, trace: bool = False, **kw):
    """inputs: full-size arrays keyed as in setup_inputs(). Returns (out, results)."""
    if "nc" not in _CACHE:
        _CACHE["nc"] = build_program()
    nc = _CACHE["nc"]

    sd16 = _to_bf16(inputs["source_distance"])   # [B, K] bf16
    # fp8(e3m4) quantization of context with per-(b,k)-row scales (rowmax
    # mapped to 14.0, under the e3m4 max of 15.5); the bf16 scales are folded
    # into the aggregation weights on-device
    ctx = np.ascontiguousarray(np.asarray(inputs["context"], np.float32))
    scales = np.maximum(np.abs(ctx).max(axis=2) / 14.0, 1e-12)    # [B, K]
    s16 = scales.astype(ml_dtypes.bfloat16)                       # [B, K]
    ctxq = (ctx * (1.0 / s16.astype(np.float32))[:, :, None]
            ).astype(ml_dtypes.float8_e3m4)                       # [B, K, D]
    consts = build_consts(
        np.asarray(inputs["kernels"], np.float32),
        np.asarray(inputs["biases"], np.float32),
        np.asarray(inputs["gate_W"], np.float32),
        np.asarray(inputs["gate_b"], np.float32),
        np.asarray(inputs["gate_weights"], np.float32),
        np.asarray(inputs["gate_bias"], np.float32),
    )

    maskpat = (np.arange(P)[:, None] // K == np.arange(ROWS)[None, :] % G)
    prow = np.arange(P) % K
    in_maps = []
    for c in range(NCORES):
        b0 = c * ROWS
        # host-side layout transforms so every device DMA run is long+contiguous
        sd_c = np.ascontiguousarray(sd16[b0:b0 + ROWS].T)                  # [K, ROWS]
        ctx_c = np.ascontiguousarray(
            ctxq[b0:b0 + ROWS].reshape(NT, NB, P, D).transpose(2, 0, 1, 3)
        ).reshape(P, NT * NB * D)
        # smask[p, g] = (p//32 == g%4) * s[b0+g, p%32]
        sm_c = np.where(maskpat, s16[b0:b0 + ROWS].T[prow], ml_dtypes.bfloat16(0))
        m = {"sd": sd_c, "ctx": ctx_c, "smask": np.ascontiguousarray(sm_c)}
        m.update(consts)
        in_maps.append(m)

    results = run_bass_kernel_spmd(nc, in_maps, core_ids=list(range(NCORES)),
                                   trace=trace, **kw)
    outs = []
    for c in range(NCORES):
        o = np.asarray(results.results[c]["out"]).astype(np.float32)
        outs.append(o.reshape(P, NT, D).transpose(1, 0, 2).reshape(ROWS, D))
    out = np.concatenate(outs, axis=0)
    return out, results


def kernel(**inputs) -> np.ndarray:
    out, _ = run(inputs)
    return out
